# revision 27
# baseline (speedup 1.0000x reference)
"""MultiHeadLocalAttention Trainium2 kernel (v6).

Sharding: data-parallel over batch B=8 across the 8 NeuronCores (one batch
element per core).

v6 design (vs v5): everything is organized around KEY-ALIGNED 128-key tiles
so each banded score is ONE wide matmul, and the AV matmul produces the
attention output TRANSPOSED (O^T, head-dim on partitions) so no PE
transposes are needed and the output projection emits y^T directly
(host un-transposes).

  - scores: per (key tile t, head h): S^T[k,q] = K_t^T Q_span, one matmul
    [64 contraction, 128 keys out, 160 q cols].  8 tiles x 6 heads = 48.
    All Q/K tiles are per-head [64, f] at partition base 0 — mixing 64-row
    weight loads at PE row 0 and row 64 back-to-back crashes TRN2.
  - exp on scalar engine (psum->SBUF), band mask via one tensor_mul with a
    replicated [128, 960] band mask.
  - AV: per (head, 512-q-chunk): oT[128, q] = sum_t Vpp2_t,h^T am_t in
    PSUM.  Vpp2[t] is [128 keys, 768]: per head 64 V columns + 64 constant
    1.0 columns, so oT rows 0:64 = O^T raw and rows 64:128 = the softmax
    denominator REPLICATED 64x.  The cls-key rank-1 term exp(s_cls) x
    [v_cls|ones] runs FIRST with start=True so no psum pre-zeroing is
    needed.
  - normalize: one vector reciprocal [64,512] (psum rows 64:128 -> SBUF)
    + one tensor_mul psum x rrect -> OT[e, q] (bf16, SBUF).
  - out proj: y^T[eo, q] = Wo^T O^T, bias added in the psum drain
    (per-partition bias).  DMA out as [E, L]; host transposes.
  - cls query (q=0) follows v5's scheme (full softmax over all keys).
  - PE p-state: a run of dummy matmuls at t=0 ramps the PE clock while
    the input DMAs stream in.
"""

import os
import sys

sys.path.insert(0, "/opt/trn_rl_repo")

import numpy as np
from contextlib import ExitStack

import concourse.bass as bass
import concourse.tile as tile
from concourse import bacc, mybir

H, D = 6, 64
WIN, PAD = 33, 16
B, L, E = 8, 1025, 384
NT = 8            # 128-key tiles covering keys 1..1024
NV = 9            # V token tiles (tokens 1..1152, zero-padded)
FP = 16           # f = abs + FP in xT/QT/KT
KTW = FP + L + 16         # 1057
XTW = FP + L + 128        # 1169
SPAN = 160                # q-span per key tile
V2W = 6 * 128             # 768: per head 64 V cols + 64 ones cols
AMW = 6 * SPAN            # 960
F32 = mybir.dt.float32
BF = mybir.dt.bfloat16
AF = mybir.ActivationFunctionType
ALU = mybir.AluOpType

N_DUMMY = 12      # PE-ramp matmuls while input DMAs run
_ORD = ["1", "21", "22", "23", "24", "3"]
LV = _ORD.index(os.environ.get("KPHASE", "3"))

# AV accumulation slices: (tile, am c0, am c1, psum col start)
AV_A = [(0, 16, 160, 0), (1, 0, 160, 112), (2, 0, 160, 240),
        (3, 0, 144, 368), (4, 0, 16, 496)]
AV_B = [(3, 144, 160, 0), (4, 16, 160, 0), (5, 0, 160, 112),
        (6, 0, 160, 240), (7, 0, 144, 368)]


def host_inputs(x_b, Wq, bq, Wk, bk, Wv, bv, Wo, bo):
    """Per-core input dict (numpy). x_b is this core's [L, E] slice."""
    import ml_dtypes
    bf = ml_dtypes.bfloat16
    scale = 1.0 / np.sqrt(np.float32(D))
    wq = np.asarray(Wq, np.float32) * scale
    bq_s = np.asarray(bq, np.float32) * scale
    bo_eff = (
        np.asarray(bv, np.float32) @ np.asarray(Wo, np.float32)
        + np.asarray(bo, np.float32)
    )

    # smalls [128, 15] f32: col h = bq head h (rows 0:64), col 6+h = bk
    # head h (rows 0:64), col 12+j = bo_eff tile j (128 rows)
    smalls = np.zeros((128, 15), np.float32)
    for h in range(6):
        smalls[0:64, h] = bq_s[64 * h:64 * h + 64]
        smalls[0:64, 6 + h] = np.asarray(bk, np.float32)[64 * h:64 * h + 64]
    for j in range(3):
        smalls[:, 12 + j] = bo_eff[128 * j:128 * j + 128]

    wv = np.asarray(Wv, np.float32)

    # band mask [128, 160]: valid iff 0 <= c - k <= 32
    k = np.arange(128)[:, None]
    c = np.arange(SPAN)[None, :]
    bandm = ((c - k >= 0) & (c - k <= 32)).astype(np.float32)

    ident = np.eye(128, dtype=np.float32)

    # x transposed + f-padded on host: [E, XTW], f = abs_token + FP
    xt = np.zeros((E, XTW), np.float32)
    xt[:, FP:FP + L] = np.asarray(x_b, np.float32).T

    return {
        "xt": np.ascontiguousarray(xt, dtype=bf),
        "wq": np.ascontiguousarray(wq, dtype=bf),
        "wk": np.ascontiguousarray(Wk, dtype=bf),
        "wv": np.ascontiguousarray(wv, dtype=bf),
        "wo": np.ascontiguousarray(Wo, dtype=bf),
        "smalls": np.ascontiguousarray(smalls),
        "ident": np.ascontiguousarray(ident, dtype=bf),
        "bandm": np.ascontiguousarray(bandm, dtype=bf),
    }


def build_program(nc):
    xd = nc.dram_tensor("xt", [E, XTW], BF, kind="ExternalInput").ap()
    wqd = nc.dram_tensor("wq", [E, E], BF, kind="ExternalInput").ap()
    wkd = nc.dram_tensor("wk", [E, E], BF, kind="ExternalInput").ap()
    wvd = nc.dram_tensor("wv", [E, E], BF, kind="ExternalInput").ap()
    wod = nc.dram_tensor("wo", [E, E], BF, kind="ExternalInput").ap()
    smd = nc.dram_tensor("smalls", [128, 15], F32, kind="ExternalInput").ap()
    idd = nc.dram_tensor("ident", [128, 128], BF, kind="ExternalInput").ap()
    bmd = nc.dram_tensor("bandm", [128, SPAN], BF, kind="ExternalInput").ap()
    outd = nc.dram_tensor("outT", [E, L], BF, kind="ExternalOutput").ap()

    with tile.TileContext(nc) as tc, ExitStack() as ctx:
        singles = ctx.enter_context(tc.tile_pool(name="singles", bufs=1))
        aepool = ctx.enter_context(tc.tile_pool(name="aepool", bufs=3))
        rpool = ctx.enter_context(tc.tile_pool(name="rpool", bufs=2))
        ypool = ctx.enter_context(tc.tile_pool(name="ypool", bufs=3))

        # ---- persistent SBUF tensors ----
        xT = [singles.tile([128, XTW], BF, tag=f"xT{j}", name=f"xT{j}")
              for j in range(3)]
        QT = [singles.tile([64, KTW], BF, tag=f"QT{h}", name=f"QT{h}")
              for h in range(6)]
        KT = [singles.tile([64, KTW], BF, tag=f"KT{h}", name=f"KT{h}")
              for h in range(6)]
        Vpp = [singles.tile([128, V2W], BF, tag=f"Vpp{t}", name=f"Vpp{t}")
               for t in range(NV)]
        OT = [singles.tile([128, L], BF, tag=f"OT{j}", name=f"OT{j}")
              for j in range(3)]
        am = [singles.tile([128, AMW], BF, tag=f"am{t}", name=f"am{t}")
              for t in range(NT)]
        mask6 = singles.tile([128, AMW], BF, tag="mask6", name="mask6")
        ecls_a = singles.tile([128, 1024], BF, tag="ecls_a", name="ecls_a")
        ecls_b = singles.tile([64, 1024], BF, tag="ecls_b", name="ecls_b")
        vcls6 = singles.tile([128, 256], BF, tag="vcls6", name="vcls6")
        vclsp = singles.tile([1, E], BF, tag="vclsp", name="vclsp")
        vclsp2 = singles.tile([1, V2W], BF, tag="vclsp2", name="vclsp2")
        cls_ab = singles.tile([128, L], BF, tag="cls_ab", name="cls_ab")
        cls_ab2 = singles.tile([64, L], BF, tag="cls_ab2", name="cls_ab2")
        acls = singles.tile([6, 1184], BF, tag="acls", name="acls")
        aclsT = singles.tile([128, 6 * NV], BF, tag="aclsT", name="aclsT")
        acls0 = singles.tile([1, 6], BF, tag="acls0", name="acls0")
        ocls_a = singles.tile([6, 512], BF, tag="ocls_a", name="ocls_a")
        ocls_b = singles.tile([6, 256], BF, tag="ocls_b", name="ocls_b")
        dw = singles.tile([128, 512], BF, tag="dw", name="dw")
        smalls_sb = singles.tile([128, 15], F32, tag="smalls", name="smalls_sb")
        ident_sb = singles.tile([128, 128], BF, tag="ident", name="ident_sb")
        bandm_sb = singles.tile([128, SPAN], BF, tag="bandm", name="bandm_sb")

        # ---- input DMAs across 3 queues; critical tiles first ----
        wsb = {}
        for nm, dr in (("wq", wqd), ("wk", wkd), ("wv", wvd), ("wo", wod)):
            wsb[nm] = [singles.tile([128, E], BF, tag=f"{nm}{ki}",
                                    name=f"{nm}{ki}") for ki in range(3)]
        for ki in range(3):
            nc.sync.dma_start(out=wsb["wq"][ki][:], in_=wqd[ki * 128:ki * 128 + 128, :])
        XH = 592
        for j in range(3):
            nc.sync.dma_start(out=xT[j][:, 0:XH], in_=xd[j * 128:(j + 1) * 128, 0:XH])
        for j in range(3):
            nc.sync.dma_start(out=xT[j][:, XH:XTW], in_=xd[j * 128:(j + 1) * 128, XH:XTW])
        for ki in range(3):
            nc.scalar.dma_start(out=wsb["wk"][ki][:], in_=wkd[ki * 128:ki * 128 + 128, :])
        for ki in range(3):
            nc.scalar.dma_start(out=wsb["wv"][ki][:], in_=wvd[ki * 128:ki * 128 + 128, :])
        for ki in range(3):
            nc.gpsimd.dma_start(out=wsb["wo"][ki][:], in_=wod[ki * 128:ki * 128 + 128, :])
        nc.gpsimd.dma_start(out=smalls_sb[:], in_=smd[:])
        nc.gpsimd.dma_start(out=ident_sb[:], in_=idd[:])
        nc.gpsimd.dma_start(out=bandm_sb[:], in_=bmd[:])

        # ---- immediate on-chip init (no DMA deps) ----
        nc.vector.memset(dw[:], 0.0)
        for h in range(6):
            nc.vector.memset(QT[h][:, 0:FP], 0.0)
            nc.vector.memset(QT[h][:, FP + L:KTW], 0.0)
        nc.vector.memset(acls[:, 0:16], 0.0)
        nc.vector.memset(acls[:, 1040:1184], 0.0)
        # ones columns of the V tiles and of vcls6 are constant 1.0
        for t in range(NV):
            nc.vector.memset(Vpp[t][:], 1.0)
        nc.vector.memset(vcls6[:], 1.0)
        nc.vector.memset(vclsp2[:], 1.0)
        # mask6 = band mask replicated per head (after bandm arrives)
        for h in range(6):
            nc.gpsimd.tensor_copy(mask6[:, SPAN * h:SPAN * h + SPAN], bandm_sb[:])

        bias_q = [smalls_sb[0:64, h:h + 1] for h in range(6)]
        bias_k = [smalls_sb[0:64, 6 + h:7 + h] for h in range(6)]
        bias_o = [smalls_sb[:, 12 + j:13 + j] for j in range(3)]

        # === single PSUM pool: big [128,1024] x3 (6 banks) + aux x2 = 8 ===
        ps = ctx.enter_context(tc.tile_pool(name="ps", bufs=1, space="PSUM"))

        def big_psum():
            return ps.tile([128, 1024], F32, tag="big", name="big", bufs=3)

        def aux_psum(p=128, dt=F32):
            return ps.tile([p, 512], dt, tag="aux", name="aux", bufs=2)

        # =========== phase 1: projections ===========
        if True:
            first = True
            for nm, dest, bias in (("wq", QT, bias_q), ("wk", KT, bias_k)):
                for j in range(3):
                    pp = big_psum()
                    if first:
                        # PE p-state ramp: harmless dummy matmuls into the
                        # first psum tile while input DMAs stream in.
                        for _ in range(N_DUMMY):
                            nc.tensor.matmul(pp[0:128, 0:512],
                                             lhsT=dw[:, 0:128], rhs=dw[:],
                                             start=True, stop=True)
                        first = False
                    for c0 in (0, 512):
                        for ki in range(3):
                            nc.tensor.matmul(
                                pp[0:128, c0:c0 + 512],
                                lhsT=wsb[nm][ki][:, 128 * j:128 * j + 128],
                                rhs=xT[ki][:, FP + c0: FP + c0 + 512],
                                start=(ki == 0), stop=(ki == 2),
                            )
                    pt_t = aux_psum()
                    for ki in range(3):
                        nc.tensor.matmul(
                            pt_t[0:128, 0:1],
                            lhsT=wsb[nm][ki][:, 128 * j:128 * j + 128],
                            rhs=xT[ki][:, FP + 1024: FP + 1025],
                            start=(ki == 0), stop=(ki == 2),
                        )
                    for par in range(2):
                        hh = 2 * j + par
                        if nm == "wq":
                            nc.scalar.activation(
                                out=dest[hh][:, FP:FP + 1024],
                                in_=pp[64 * par:64 * par + 64, 0:1024],
                                func=AF.Identity, bias=bias[hh], scale=1.0,
                            )
                            nc.scalar.activation(
                                out=dest[hh][:, FP + 1024:FP + 1025],
                                in_=pt_t[64 * par:64 * par + 64, 0:1],
                                func=AF.Identity, bias=bias[hh], scale=1.0,
                            )
                        else:
                            nc.vector.tensor_scalar_add(
                                dest[hh][:, FP:FP + 1024],
                                pp[64 * par:64 * par + 64, 0:1024], bias[hh],
                            )
                            nc.vector.tensor_scalar_add(
                                dest[hh][:, FP + 1024:FP + 1025],
                                pt_t[64 * par:64 * par + 64, 0:1], bias[hh],
                            )

            # vcls = V row of token 0
            pvc = aux_psum()
            for ki in range(3):
                nc.tensor.matmul(
                    pvc[0:1, 0:E], lhsT=xT[ki][:, FP:FP + 1],
                    rhs=wsb["wv"][ki][:], start=(ki == 0), stop=(ki == 2),
                )
            nc.vector.tensor_copy(vclsp[:], pvc[0:1, 0:E])
            # scatter vcls to partitions 32*(h%4), cols 128*(h//4)[+0:64]
            for h in range(6):
                nc.sync.dma_start(
                    out=vcls6[32 * (h % 4):32 * (h % 4) + 1,
                              128 * (h // 4):128 * (h // 4) + 64],
                    in_=vclsp[0:1, 64 * h:64 * h + 64],
                )
            # vclsp2: token-0 V row in Vpp2 layout (V blocks strided)
            nc.sync.dma_start(
                out=vclsp2[0:1, :].rearrange("p (h c) -> p h c", h=6)[:, :, 0:64],
                in_=vclsp[0:1, :].rearrange("p (h c) -> p h c", h=6),
            )
            # V tiles: tokens [128t+1, 128t+129); V cols land strided
            # (64 per head), ones cols stay at the memset 1.0
            for t in range(NV):
                pv = aux_psum()
                for ki in range(3):
                    nc.tensor.matmul(
                        pv[:, 0:E],
                        lhsT=xT[ki][:, FP + 1 + 128 * t: FP + 129 + 128 * t],
                        rhs=wsb["wv"][ki][:],
                        start=(ki == 0), stop=(ki == 2),
                    )
                nc.vector.tensor_copy(
                    Vpp[t][:].rearrange("p (h c) -> p h c", h=6)[:, :, 0:64],
                    pv[:, 0:E].rearrange("p (h c) -> p h c", h=6),
                )

        # =========== phase 2: scores + cls ===========
        if LV >= 1:
            # --- cls-query scores: s[h, k] for q=0 over all keys ---
            CCH = [(0, 512), (512, 512), (1024, 1)]
            for c0, w in CCH:
                pa = aux_psum()
                pb = aux_psum()
                # init full tiles via zero outer-product (dw is zeros)
                nc.tensor.matmul(pa[0:128, 0:w], lhsT=dw[0:1, 0:128],
                                 rhs=dw[0:1, 0:w], start=True, stop=True)
                nc.tensor.matmul(pb[0:128, 0:w], lhsT=dw[0:1, 0:128],
                                 rhs=dw[0:1, 0:w], start=True, stop=True)
                for h in range(6):
                    dstp = pa if h < 4 else pb
                    base = 32 * (h % 4)
                    nc.tensor.matmul(
                        dstp[base:base + 1, 0:w],
                        lhsT=QT[h][0:64, FP:FP + 1],
                        rhs=KT[h][0:64, FP + c0:FP + c0 + w],
                        start=True, stop=True, tile_position=(0, base),
                    )
                nc.scalar.activation(out=cls_ab[:, c0:c0 + w],
                                     in_=pa[:, 0:w], func=AF.Exp)
                nc.scalar.activation(out=cls_ab2[:, c0:c0 + w],
                                     in_=pb[0:64, 0:w], func=AF.Exp)

            # --- cls-KEY scores: s_cls[h, q] for q=1..1024 ---
            for ci in (range(2) if LV >= 2 else []):
                pe_a = aux_psum()
                pe_b = aux_psum(p=64)
                qs0 = FP + 1 + 512 * ci
                nc.tensor.matmul(pe_a[0:128, 0:512], lhsT=dw[0:1, 0:128],
                                 rhs=dw[0:1, 0:512], start=True, stop=True)
                nc.tensor.matmul(pe_b[0:64, 0:512], lhsT=dw[0:1, 0:64],
                                 rhs=dw[0:1, 0:512], start=True, stop=True)
                for h in range(6):
                    dstp = pe_a if h < 4 else pe_b
                    base = 32 * (h % 4)
                    nc.tensor.matmul(
                        dstp[base:base + 1, 0:512],
                        lhsT=KT[h][0:64, FP:FP + 1],
                        rhs=QT[h][0:64, qs0:qs0 + 512],
                        start=True, stop=True, tile_position=(0, base),
                    )
                nc.scalar.activation(
                    out=ecls_a[:, 512 * ci:512 * ci + 512],
                    in_=pe_a[:], func=AF.Exp)
                nc.scalar.activation(
                    out=ecls_b[:, 512 * ci:512 * ci + 512],
                    in_=pe_b[:], func=AF.Exp)

            # --- banded scores per key tile ---
            for t in (range(NT) if LV >= 3 else []):
                scp = big_psum()
                k0 = FP + 1 + 128 * t
                q0 = FP - 15 + 128 * t
                for h in range(6):
                    off = 512 * (h // 3) + SPAN * (h % 3)
                    nc.tensor.matmul(
                        scp[:, off:off + SPAN],
                        lhsT=KT[h][0:64, k0:k0 + 128],
                        rhs=QT[h][0:64, q0:q0 + SPAN],
                        start=True, stop=True,
                    )
                a_e = aepool.tile([128, AMW], BF, tag="a_e", name="a_e")
                nc.scalar.activation(out=a_e[:, 0:480], in_=scp[:, 0:480],
                                     func=AF.Exp)
                nc.scalar.activation(out=a_e[:, 480:960], in_=scp[:, 512:992],
                                     func=AF.Exp)
                nc.gpsimd.tensor_mul(am[t][:], a_e[:], mask6[:])

            # --- cls-query: gather rows, transpose, AV ---
            for h in (range(6) if LV >= 4 else []):
                srct = cls_ab if h < 4 else cls_ab2
                nc.sync.dma_start(
                    out=acls[h:h + 1, 15:15 + L],
                    in_=srct[32 * (h % 4):32 * (h % 4) + 1, 0:L],
                )
            for t in (range(NV) if LV >= 4 else []):
                pt = aux_psum(dt=BF)
                nc.tensor.transpose(
                    pt[0:128, 0:6], acls[0:6, 128 * t + 16:128 * t + 144],
                    ident_sb[0:6, 0:6],
                )
                nc.vector.tensor_copy(aclsT[:, 6 * t:6 * t + 6], pt[0:128, 0:6])
        if LV >= 4:
            pt0 = aux_psum(dt=BF)
            nc.tensor.transpose(pt0[0:1, 0:6], acls[0:6, 15:16],
                                ident_sb[0:6, 0:6])
            nc.vector.tensor_copy(acls0[:], pt0[0:1, 0:6])

            # cls AV over Vpp2 (two col halves); sums ride any ones col
            poc_a = aux_psum()
            poc_b = aux_psum(p=64)
            nc.tensor.matmul(poc_a[0:6, 0:512], lhsT=acls0[0:1, :],
                             rhs=vclsp2[0:1, 0:512], start=True, stop=False)
            nc.tensor.matmul(poc_b[0:6, 0:256], lhsT=acls0[0:1, :],
                             rhs=vclsp2[0:1, 512:768], start=True, stop=False)
            for t in range(NV):
                nc.tensor.matmul(
                    poc_a[0:6, 0:512], lhsT=aclsT[:, 6 * t:6 * t + 6],
                    rhs=Vpp[t][:, 0:512], start=False, stop=(t == NV - 1),
                )
                nc.tensor.matmul(
                    poc_b[0:6, 0:256], lhsT=aclsT[:, 6 * t:6 * t + 6],
                    rhs=Vpp[t][:, 512:768], start=False, stop=(t == NV - 1),
                )
            rc = singles.tile([6, 1], F32, tag="rc", name="rc")
            nc.vector.reciprocal(rc[:], poc_a[0:6, 64:65])
            nc.vector.tensor_scalar_mul(ocls_a[:], poc_a[0:6, 0:512], rc[:])
            nc.vector.tensor_scalar_mul(ocls_b[:], poc_b[0:6, 0:256], rc[:])
            for h in range(6):
                srco = ocls_a if h < 4 else ocls_b
                c0 = 128 * h if h < 4 else 128 * (h - 4)
                nc.sync.dma_start(
                    out=OT[h // 2][64 * (h % 2):64 * (h % 2) + 64, 0:1],
                    in_=srco[h:h + 1, c0:c0 + 64],
                )

        # =========== phase 3: AV + normalize + out proj ===========
        if LV >= 5:
            def av_chunk(ci, slices):
                for hp in range(3):
                    pair = big_psum()
                    for par in range(2):
                        h = 2 * hp + par
                        po = pair[:, 512 * par:512 * par + 512]
                        esrc = ecls_a if h < 4 else ecls_b
                        erow = 32 * (h % 4)
                        # cls-key rank-1 first: start=True zeroes the half
                        nc.tensor.matmul(
                            po[0:128, 0:512],
                            lhsT=vcls6[erow:erow + 1,
                                       128 * (h // 4):128 * (h // 4) + 128],
                            rhs=esrc[erow:erow + 1, 512 * ci:512 * ci + 512],
                            start=True, stop=False, tile_position=(erow, 0),
                        )
                        for si, (t, c0, c1, p0) in enumerate(slices):
                            nc.tensor.matmul(
                                po[0:128, p0:p0 + (c1 - c0)],
                                lhsT=Vpp[t][:, 128 * h:128 * h + 128],
                                rhs=am[t][:, SPAN * h + c0:SPAN * h + c1],
                                start=False, stop=(si == len(slices) - 1),
                            )
                        # normalize: r = 1/sums (replicated on rows 64:128)
                        rrect = rpool.tile([64, 512], F32, tag="rrect",
                                           name="rrect")
                        nc.vector.reciprocal(rrect[:], po[64:128, 0:512])
                        nc.vector.tensor_mul(
                            OT[h // 2][64 * (h % 2):64 * (h % 2) + 64,
                                       1 + 512 * ci:513 + 512 * ci],
                            po[0:64, 0:512], rrect[:],
                        )

            def out_proj(ci, c0, w):
                for j in range(3):
                    py = aux_psum()
                    for ki in range(3):
                        nc.tensor.matmul(
                            py[0:128, 0:w],
                            lhsT=wsb["wo"][ki][:, 128 * j:128 * j + 128],
                            rhs=OT[ki][:, c0:c0 + w],
                            start=(ki == 0), stop=(ki == 2),
                        )
                    ysb = ypool.tile([128, 512], BF, tag="ysb", name="ysb")
                    nc.scalar.activation(out=ysb[0:128, 0:w], in_=py[0:128, 0:w],
                                         func=AF.Identity, bias=bias_o[j],
                                         scale=1.0)
                    nc.sync.dma_start(out=outd[128 * j:128 * j + 128, c0:c0 + w],
                                      in_=ysb[0:128, 0:w])

            av_chunk(0, AV_A)
            out_proj(0, 0, 512)
            av_chunk(1, AV_B)
            out_proj(1, 512, 512)
            out_proj(2, 1024, 1)
        if LV < 5:
            for j in range(3):
                nc.sync.dma_start(out=outd[128 * j:128 * j + 128, 0:L],
                                  in_=xT[j][:, FP:FP + L])

    nc.compile()
    return nc


_CACHE = {}


def get_nc():
    if "nc" not in _CACHE:
        nc = bacc.Bacc("TRN2", target_bir_lowering=False, debug=False)
        _CACHE["nc"] = build_program(nc)
    return _CACHE["nc"]


def kernel(x, Wq, bq, Wk, bk, Wv, bv, Wo, bo, _trace=False):
    from concourse.bass_utils import run_bass_kernel_spmd

    x = np.asarray(x)
    in_maps = [
        host_inputs(x[b], Wq, bq, Wk, bk, Wv, bv, Wo, bo) for b in range(B)
    ]
    nc = get_nc()
    res = run_bass_kernel_spmd(nc, in_maps, core_ids=list(range(8)), trace=_trace)
    out = np.stack(
        [np.asarray(res.results[b]["outT"], dtype=np.float32).T for b in range(B)],
        axis=0,
    )
    if _trace:
        return out, res
    return out


# revision 30
# speedup vs baseline: 1.2640x; 1.2640x over previous
"""MultiHeadLocalAttention Trainium2 kernel (v6).

Sharding: data-parallel over batch B=8 across the 8 NeuronCores (one batch
element per core).

v6 design (vs v5): everything is organized around KEY-ALIGNED 128-key tiles
so each banded score is ONE wide matmul, and the AV matmul produces the
attention output TRANSPOSED (O^T, head-dim on partitions) so no PE
transposes are needed and the output projection emits y^T directly
(host un-transposes).

  - scores: per (key tile t, head h): S^T[k,q] = K_t^T Q_span, one matmul
    [64 contraction, 128 keys out, 160 q cols].  8 tiles x 6 heads = 48.
    All Q/K tiles are per-head [64, f] at partition base 0 — mixing 64-row
    weight loads at PE row 0 and row 64 back-to-back crashes TRN2.
  - exp on scalar engine (psum->SBUF), band mask via one tensor_mul with a
    replicated [128, 960] band mask.
  - AV: per (head, 512-q-chunk): oT[128, q] = sum_t Vpp2_t,h^T am_t in
    PSUM.  Vpp2[t] is [128 keys, 768]: per head 64 V columns + 64 constant
    1.0 columns, so oT rows 0:64 = O^T raw and rows 64:128 = the softmax
    denominator REPLICATED 64x.  The cls-key rank-1 term exp(s_cls) x
    [v_cls|ones] runs FIRST with start=True so no psum pre-zeroing is
    needed.
  - normalize: one vector reciprocal [64,512] (psum rows 64:128 -> SBUF)
    + one tensor_mul psum x rrect -> OT[e, q] (bf16, SBUF).
  - out proj: y^T[eo, q] = Wo^T O^T, bias added in the psum drain
    (per-partition bias).  DMA out as [E, L]; host transposes.
  - cls query (q=0) follows v5's scheme (full softmax over all keys).
  - PE p-state: a run of dummy matmuls at t=0 ramps the PE clock while
    the input DMAs stream in.
"""

import os
import sys

sys.path.insert(0, "/opt/trn_rl_repo")

import numpy as np
from contextlib import ExitStack

import concourse.bass as bass
import concourse.tile as tile
from concourse import bacc, mybir

H, D = 6, 64
WIN, PAD = 33, 16
B, L, E = 8, 1025, 384
NT = 8            # 128-key tiles covering keys 1..1024
NV = 9            # V token tiles (tokens 1..1152, zero-padded)
FP = 16           # f = abs + FP in xT/QT/KT
KTW = FP + L + 16         # 1057
XTW = FP + L + 128        # 1169
SPAN = 160                # q-span per key tile
V2W = 6 * 128             # 768: per head 64 V cols + 64 ones cols
AMW = 6 * SPAN            # 960
F32 = mybir.dt.float32
BF = mybir.dt.bfloat16
AF = mybir.ActivationFunctionType
ALU = mybir.AluOpType

N_DUMMY = 12      # PE-ramp matmuls while input DMAs run
_ORD = ["1", "21", "22", "23", "24", "3"]
LV = _ORD.index(os.environ.get("KPHASE", "3"))

# AV accumulation slices: (tile, am c0, am c1, psum col start)
AV_A = [(0, 16, 160, 0), (1, 0, 160, 112), (2, 0, 160, 240),
        (3, 0, 144, 368), (4, 0, 16, 496)]
AV_B = [(3, 144, 160, 0), (4, 16, 160, 0), (5, 0, 160, 112),
        (6, 0, 160, 240), (7, 0, 144, 368)]


def host_inputs(x_b, Wq, bq, Wk, bk, Wv, bv, Wo, bo):
    """Per-core input dict (numpy). x_b is this core's [L, E] slice."""
    import ml_dtypes
    bf = ml_dtypes.bfloat16
    scale = 1.0 / np.sqrt(np.float32(D))
    wq = np.asarray(Wq, np.float32) * scale
    bq_s = np.asarray(bq, np.float32) * scale
    bo_eff = (
        np.asarray(bv, np.float32) @ np.asarray(Wo, np.float32)
        + np.asarray(bo, np.float32)
    )

    # smalls [128, 15] f32: col h = bq head h (rows 0:64), col 6+h = bk
    # head h (rows 0:64), col 12+j = bo_eff tile j (128 rows)
    smalls = np.zeros((128, 15), np.float32)
    for h in range(6):
        smalls[0:64, h] = bq_s[64 * h:64 * h + 64]
        smalls[0:64, 6 + h] = np.asarray(bk, np.float32)[64 * h:64 * h + 64]
    for j in range(3):
        smalls[:, 12 + j] = bo_eff[128 * j:128 * j + 128]

    wv = np.asarray(Wv, np.float32)

    # band mask [128, 160]: valid iff 0 <= c - k <= 32
    k = np.arange(128)[:, None]
    c = np.arange(SPAN)[None, :]
    bandm = ((c - k >= 0) & (c - k <= 32)).astype(np.float32)

    ident = np.eye(128, dtype=np.float32)

    # x transposed + f-padded on host: [E, XTW], f = abs_token + FP
    xt = np.zeros((E, XTW), np.float32)
    xt[:, FP:FP + L] = np.asarray(x_b, np.float32).T

    return {
        "xt": np.ascontiguousarray(xt, dtype=bf),
        "wq": np.ascontiguousarray(wq, dtype=bf),
        "wk": np.ascontiguousarray(Wk, dtype=bf),
        "wv": np.ascontiguousarray(wv, dtype=bf),
        "wo": np.ascontiguousarray(Wo, dtype=bf),
        "smalls": np.ascontiguousarray(smalls),
        "ident": np.ascontiguousarray(ident, dtype=bf),
        "bandm": np.ascontiguousarray(bandm, dtype=bf),
    }


def build_program(nc):
    xd = nc.dram_tensor("xt", [E, XTW], BF, kind="ExternalInput").ap()
    wqd = nc.dram_tensor("wq", [E, E], BF, kind="ExternalInput").ap()
    wkd = nc.dram_tensor("wk", [E, E], BF, kind="ExternalInput").ap()
    wvd = nc.dram_tensor("wv", [E, E], BF, kind="ExternalInput").ap()
    wod = nc.dram_tensor("wo", [E, E], BF, kind="ExternalInput").ap()
    smd = nc.dram_tensor("smalls", [128, 15], F32, kind="ExternalInput").ap()
    idd = nc.dram_tensor("ident", [128, 128], BF, kind="ExternalInput").ap()
    bmd = nc.dram_tensor("bandm", [128, SPAN], BF, kind="ExternalInput").ap()
    outd = nc.dram_tensor("outT", [E, L], BF, kind="ExternalOutput").ap()

    with tile.TileContext(nc) as tc, ExitStack() as ctx:
        singles = ctx.enter_context(tc.tile_pool(name="singles", bufs=1))
        aepool = ctx.enter_context(tc.tile_pool(name="aepool", bufs=3))
        rpool = ctx.enter_context(tc.tile_pool(name="rpool", bufs=2))
        ypool = ctx.enter_context(tc.tile_pool(name="ypool", bufs=3))

        # ---- persistent SBUF tensors ----
        xT = [singles.tile([128, XTW], BF, tag=f"xT{j}", name=f"xT{j}")
              for j in range(3)]
        QT = [singles.tile([64, KTW], BF, tag=f"QT{h}", name=f"QT{h}")
              for h in range(6)]
        KT = [singles.tile([64, KTW], BF, tag=f"KT{h}", name=f"KT{h}")
              for h in range(6)]
        Vpp = [singles.tile([128, V2W], BF, tag=f"Vpp{t}", name=f"Vpp{t}")
               for t in range(NV)]
        OT = [singles.tile([128, L], BF, tag=f"OT{j}", name=f"OT{j}")
              for j in range(3)]
        am = [singles.tile([128, AMW], BF, tag=f"am{t}", name=f"am{t}")
              for t in range(NT)]
        mask6 = singles.tile([128, AMW], BF, tag="mask6", name="mask6")
        ecls_a = singles.tile([128, 1024], BF, tag="ecls_a", name="ecls_a")
        ecls_b = singles.tile([64, 1024], BF, tag="ecls_b", name="ecls_b")
        vcls6 = singles.tile([128, 256], BF, tag="vcls6", name="vcls6")
        vclsp = singles.tile([1, E], BF, tag="vclsp", name="vclsp")
        vclsp2 = singles.tile([1, V2W], BF, tag="vclsp2", name="vclsp2")
        cls_ab = singles.tile([128, L], BF, tag="cls_ab", name="cls_ab")
        cls_ab2 = singles.tile([64, L], BF, tag="cls_ab2", name="cls_ab2")
        acls = singles.tile([6, 1184], BF, tag="acls", name="acls")
        aclsT = singles.tile([128, 6 * NV], BF, tag="aclsT", name="aclsT")
        acls0 = singles.tile([1, 6], BF, tag="acls0", name="acls0")
        ocls_a = singles.tile([6, 512], BF, tag="ocls_a", name="ocls_a")
        ocls_b = singles.tile([6, 256], BF, tag="ocls_b", name="ocls_b")
        dw = singles.tile([128, 512], BF, tag="dw", name="dw")
        smalls_sb = singles.tile([128, 15], F32, tag="smalls", name="smalls_sb")
        ident_sb = singles.tile([128, 128], BF, tag="ident", name="ident_sb")
        bandm_sb = singles.tile([128, SPAN], BF, tag="bandm", name="bandm_sb")

        # ---- input DMAs across 3 queues; critical tiles first ----
        wsb = {}
        for nm, dr in (("wq", wqd), ("wk", wkd), ("wv", wvd), ("wo", wod)):
            wsb[nm] = [singles.tile([128, E], BF, tag=f"{nm}{ki}",
                                    name=f"{nm}{ki}") for ki in range(3)]
        for ki in range(3):
            nc.sync.dma_start(out=wsb["wq"][ki][:], in_=wqd[ki * 128:ki * 128 + 128, :])
        XH = 592
        for j in range(3):
            nc.sync.dma_start(out=xT[j][:, 0:XH], in_=xd[j * 128:(j + 1) * 128, 0:XH])
        for j in range(3):
            nc.sync.dma_start(out=xT[j][:, XH:XTW], in_=xd[j * 128:(j + 1) * 128, XH:XTW])
        for ki in range(3):
            nc.scalar.dma_start(out=wsb["wk"][ki][:], in_=wkd[ki * 128:ki * 128 + 128, :])
        for ki in range(3):
            nc.scalar.dma_start(out=wsb["wv"][ki][:], in_=wvd[ki * 128:ki * 128 + 128, :])
        for ki in range(3):
            nc.gpsimd.dma_start(out=wsb["wo"][ki][:], in_=wod[ki * 128:ki * 128 + 128, :])
        nc.gpsimd.dma_start(out=smalls_sb[:], in_=smd[:])
        nc.gpsimd.dma_start(out=ident_sb[:], in_=idd[:])
        nc.gpsimd.dma_start(out=bandm_sb[:], in_=bmd[:])

        # ---- immediate on-chip init (no DMA deps) ----
        nc.vector.memset(dw[:], 0.0)
        for h in range(6):
            nc.vector.memset(QT[h][:, 0:FP], 0.0)
            nc.vector.memset(QT[h][:, FP + L:KTW], 0.0)
        nc.vector.memset(acls[:, 0:16], 0.0)
        nc.vector.memset(acls[:, 1040:1184], 0.0)
        # ones columns of the V tiles and of vcls6 are constant 1.0
        for t in range(NV):
            nc.vector.memset(Vpp[t][:], 1.0)
        nc.vector.memset(vcls6[:], 1.0)
        nc.vector.memset(vclsp2[:], 1.0)
        # mask6 = band mask replicated per head (after bandm arrives)
        for h in range(6):
            nc.gpsimd.tensor_copy(mask6[:, SPAN * h:SPAN * h + SPAN], bandm_sb[:])

        bias_q = [smalls_sb[0:64, h:h + 1] for h in range(6)]
        bias_k = [smalls_sb[0:64, 6 + h:7 + h] for h in range(6)]
        bias_o = [smalls_sb[:, 12 + j:13 + j] for j in range(3)]

        # === single PSUM pool: big [128,1024] x3 (6 banks) + aux x2 = 8 ===
        ps = ctx.enter_context(tc.tile_pool(name="ps", bufs=1, space="PSUM"))

        def big_psum():
            return ps.tile([128, 1024], F32, tag="big", name="big", bufs=3)

        def aux_psum(p=128, dt=F32):
            return ps.tile([p, 512], dt, tag="aux", name="aux", bufs=2)

        # =========== phase 1: projections ===========
        if True:
            first = True
            for nm, dest, bias in (("wq", QT, bias_q), ("wk", KT, bias_k)):
                for j in range(3):
                    pp = big_psum()
                    if first:
                        # PE p-state ramp: harmless dummy matmuls into the
                        # first psum tile while input DMAs stream in.
                        for _ in range(N_DUMMY):
                            nc.tensor.matmul(pp[0:128, 0:512],
                                             lhsT=dw[:, 0:128], rhs=dw[:],
                                             start=True, stop=True)
                        first = False
                    for c0 in (0, 512):
                        for ki in range(3):
                            nc.tensor.matmul(
                                pp[0:128, c0:c0 + 512],
                                lhsT=wsb[nm][ki][:, 128 * j:128 * j + 128],
                                rhs=xT[ki][:, FP + c0: FP + c0 + 512],
                                start=(ki == 0), stop=(ki == 2),
                            )
                    pt_t = aux_psum()
                    for ki in range(3):
                        nc.tensor.matmul(
                            pt_t[0:128, 0:1],
                            lhsT=wsb[nm][ki][:, 128 * j:128 * j + 128],
                            rhs=xT[ki][:, FP + 1024: FP + 1025],
                            start=(ki == 0), stop=(ki == 2),
                        )
                    for par in range(2):
                        hh = 2 * j + par
                        if nm == "wq":
                            nc.scalar.activation(
                                out=dest[hh][:, FP:FP + 1024],
                                in_=pp[64 * par:64 * par + 64, 0:1024],
                                func=AF.Identity, bias=bias[hh], scale=1.0,
                            )
                            nc.scalar.activation(
                                out=dest[hh][:, FP + 1024:FP + 1025],
                                in_=pt_t[64 * par:64 * par + 64, 0:1],
                                func=AF.Identity, bias=bias[hh], scale=1.0,
                            )
                        else:
                            nc.vector.tensor_scalar_add(
                                dest[hh][:, FP:FP + 1024],
                                pp[64 * par:64 * par + 64, 0:1024], bias[hh],
                            )
                            nc.vector.tensor_scalar_add(
                                dest[hh][:, FP + 1024:FP + 1025],
                                pt_t[64 * par:64 * par + 64, 0:1], bias[hh],
                            )

            # vcls = V row of token 0
            pvc = aux_psum()
            for ki in range(3):
                nc.tensor.matmul(
                    pvc[0:1, 0:E], lhsT=xT[ki][:, FP:FP + 1],
                    rhs=wsb["wv"][ki][:], start=(ki == 0), stop=(ki == 2),
                )
            nc.vector.tensor_copy(vclsp[:], pvc[0:1, 0:E])
            # scatter vcls to partitions 32*(h%4), cols 128*(h//4)[+0:64]
            for h in range(6):
                nc.sync.dma_start(
                    out=vcls6[32 * (h % 4):32 * (h % 4) + 1,
                              128 * (h // 4) + 64:128 * (h // 4) + 128],
                    in_=vclsp[0:1, 64 * h:64 * h + 64],
                )
            # vclsp2: token-0 V row in Vpp2 layout (V blocks strided)
            nc.sync.dma_start(
                out=vclsp2[0:1, :].rearrange("p (h c) -> p h c", h=6)[:, :, 64:128],
                in_=vclsp[0:1, :].rearrange("p (h c) -> p h c", h=6),
            )
            # V tiles: tokens [128t+1, 128t+129); V cols land strided
            # (64 per head), ones cols stay at the memset 1.0
            for t in range(NV):
                pv = aux_psum()
                for ki in range(3):
                    nc.tensor.matmul(
                        pv[:, 0:E],
                        lhsT=xT[ki][:, FP + 1 + 128 * t: FP + 129 + 128 * t],
                        rhs=wsb["wv"][ki][:],
                        start=(ki == 0), stop=(ki == 2),
                    )
                nc.vector.tensor_copy(
                    Vpp[t][:].rearrange("p (h c) -> p h c", h=6)[:, :, 64:128],
                    pv[:, 0:E].rearrange("p (h c) -> p h c", h=6),
                )

        # =========== phase 2: scores + cls ===========
        if LV >= 1:
            # --- cls-query scores: s[h, k] for q=0 over all keys ---
            CCH = [(0, 512), (512, 512), (1024, 1)]
            for c0, w in CCH:
                pa = aux_psum()
                pb = aux_psum()
                # init full tiles via zero outer-product (dw is zeros)
                nc.tensor.matmul(pa[0:128, 0:w], lhsT=dw[0:1, 0:128],
                                 rhs=dw[0:1, 0:w], start=True, stop=True)
                nc.tensor.matmul(pb[0:128, 0:w], lhsT=dw[0:1, 0:128],
                                 rhs=dw[0:1, 0:w], start=True, stop=True)
                for h in range(6):
                    dstp = pa if h < 4 else pb
                    base = 32 * (h % 4)
                    nc.tensor.matmul(
                        dstp[base:base + 1, 0:w],
                        lhsT=QT[h][0:64, FP:FP + 1],
                        rhs=KT[h][0:64, FP + c0:FP + c0 + w],
                        start=True, stop=True, tile_position=(0, base),
                    )
                nc.scalar.activation(out=cls_ab[:, c0:c0 + w],
                                     in_=pa[:, 0:w], func=AF.Exp)
                nc.scalar.activation(out=cls_ab2[:, c0:c0 + w],
                                     in_=pb[0:64, 0:w], func=AF.Exp)

            # --- cls-KEY scores: s_cls[h, q] for q=1..1024 ---
            for ci in (range(2) if LV >= 2 else []):
                pe_a = aux_psum()
                pe_b = aux_psum(p=64)
                qs0 = FP + 1 + 512 * ci
                nc.tensor.matmul(pe_a[0:128, 0:512], lhsT=dw[0:1, 0:128],
                                 rhs=dw[0:1, 0:512], start=True, stop=True)
                nc.tensor.matmul(pe_b[0:64, 0:512], lhsT=dw[0:1, 0:64],
                                 rhs=dw[0:1, 0:512], start=True, stop=True)
                for h in range(6):
                    dstp = pe_a if h < 4 else pe_b
                    base = 32 * (h % 4)
                    nc.tensor.matmul(
                        dstp[base:base + 1, 0:512],
                        lhsT=KT[h][0:64, FP:FP + 1],
                        rhs=QT[h][0:64, qs0:qs0 + 512],
                        start=True, stop=True, tile_position=(0, base),
                    )
                nc.scalar.activation(
                    out=ecls_a[:, 512 * ci:512 * ci + 512],
                    in_=pe_a[:], func=AF.Exp)
                nc.scalar.activation(
                    out=ecls_b[:, 512 * ci:512 * ci + 512],
                    in_=pe_b[:], func=AF.Exp)

            # --- banded scores per key tile ---
            for t in (range(NT) if LV >= 3 else []):
                scp = big_psum()
                k0 = FP + 1 + 128 * t
                q0 = FP - 15 + 128 * t
                for h in range(6):
                    off = 512 * (h // 3) + SPAN * (h % 3)
                    nc.tensor.matmul(
                        scp[:, off:off + SPAN],
                        lhsT=KT[h][0:64, k0:k0 + 128],
                        rhs=QT[h][0:64, q0:q0 + SPAN],
                        start=True, stop=True,
                    )
                a_e = aepool.tile([128, AMW], BF, tag="a_e", name="a_e")
                nc.scalar.activation(out=a_e[:, 0:480], in_=scp[:, 0:480],
                                     func=AF.Exp)
                nc.scalar.activation(out=a_e[:, 480:960], in_=scp[:, 512:992],
                                     func=AF.Exp)
                nc.gpsimd.tensor_mul(am[t][:], a_e[:], mask6[:])

            # --- cls-query: gather rows, transpose, AV ---
            for h in (range(6) if LV >= 4 else []):
                srct = cls_ab if h < 4 else cls_ab2
                nc.sync.dma_start(
                    out=acls[h:h + 1, 15:15 + L],
                    in_=srct[32 * (h % 4):32 * (h % 4) + 1, 0:L],
                )
            for t in (range(NV) if LV >= 4 else []):
                pt = aux_psum(dt=BF)
                nc.tensor.transpose(
                    pt[0:128, 0:6], acls[0:6, 128 * t + 16:128 * t + 144],
                    ident_sb[0:6, 0:6],
                )
                nc.vector.tensor_copy(aclsT[:, 6 * t:6 * t + 6], pt[0:128, 0:6])
        if LV >= 4:
            pt0 = aux_psum(dt=BF)
            nc.tensor.transpose(pt0[0:1, 0:6], acls[0:6, 15:16],
                                ident_sb[0:6, 0:6])
            nc.vector.tensor_copy(acls0[:], pt0[0:1, 0:6])

            # cls AV over Vpp2 (two col halves); sums ride any ones col
            poc_a = aux_psum()
            poc_b = aux_psum(p=64)
            nc.tensor.matmul(poc_a[0:6, 0:512], lhsT=acls0[0:1, :],
                             rhs=vclsp2[0:1, 0:512], start=True, stop=False)
            nc.tensor.matmul(poc_b[0:6, 0:256], lhsT=acls0[0:1, :],
                             rhs=vclsp2[0:1, 512:768], start=True, stop=False)
            for t in range(NV):
                nc.tensor.matmul(
                    poc_a[0:6, 0:512], lhsT=aclsT[:, 6 * t:6 * t + 6],
                    rhs=Vpp[t][:, 0:512], start=False, stop=(t == NV - 1),
                )
                nc.tensor.matmul(
                    poc_b[0:6, 0:256], lhsT=aclsT[:, 6 * t:6 * t + 6],
                    rhs=Vpp[t][:, 512:768], start=False, stop=(t == NV - 1),
                )
            rc = singles.tile([6, 1], F32, tag="rc", name="rc")
            nc.vector.reciprocal(rc[:], poc_a[0:6, 0:1])
            nc.vector.tensor_scalar_mul(ocls_a[:], poc_a[0:6, 0:512], rc[:])
            nc.vector.tensor_scalar_mul(ocls_b[:], poc_b[0:6, 0:256], rc[:])
            for h in range(6):
                srco = ocls_a if h < 4 else ocls_b
                c0 = (128 * h if h < 4 else 128 * (h - 4)) + 64
                nc.sync.dma_start(
                    out=OT[h // 2][64 * (h % 2):64 * (h % 2) + 64, 0:1],
                    in_=srco[h:h + 1, c0:c0 + 64],
                )

        # =========== phase 3: AV + normalize + out proj ===========
        if LV >= 5:
            def av_chunk(ci, slices):
                for hp in range(3):
                    pair = big_psum()
                    for par in range(2):
                        h = 2 * hp + par
                        po = pair[:, 512 * par:512 * par + 512]
                        esrc = ecls_a if h < 4 else ecls_b
                        erow = 32 * (h % 4)
                        # cls-key rank-1 first: start=True zeroes the half
                        nc.tensor.matmul(
                            po[0:128, 0:512],
                            lhsT=vcls6[erow:erow + 1,
                                       128 * (h // 4):128 * (h // 4) + 128],
                            rhs=esrc[erow:erow + 1, 512 * ci:512 * ci + 512],
                            start=True, stop=False, tile_position=(erow, 0),
                        )
                        for si, (t, c0, c1, p0) in enumerate(slices):
                            nc.tensor.matmul(
                                po[0:128, p0:p0 + (c1 - c0)],
                                lhsT=Vpp[t][:, 128 * h:128 * h + 128],
                                rhs=am[t][:, SPAN * h + c0:SPAN * h + c1],
                                start=False, stop=(si == len(slices) - 1),
                            )
                        # normalize: r = 1/sums (replicated on rows 64:128)
                        rrect = rpool.tile([64, 512], F32, tag="rrect",
                                           name="rrect")
                        nc.vector.reciprocal_approx_fast(
                            out=rrect[:], in_=po[0:64, 0:512])
                        nc.vector.tensor_mul(
                            OT[h // 2][64 * (h % 2):64 * (h % 2) + 64,
                                       1 + 512 * ci:513 + 512 * ci],
                            po[64:128, 0:512], rrect[:],
                        )

            def out_proj(ci, c0, w):
                for j in range(3):
                    py = aux_psum()
                    for ki in range(3):
                        nc.tensor.matmul(
                            py[0:128, 0:w],
                            lhsT=wsb["wo"][ki][:, 128 * j:128 * j + 128],
                            rhs=OT[ki][:, c0:c0 + w],
                            start=(ki == 0), stop=(ki == 2),
                        )
                    ysb = ypool.tile([128, 512], BF, tag="ysb", name="ysb")
                    nc.scalar.activation(out=ysb[0:128, 0:w], in_=py[0:128, 0:w],
                                         func=AF.Identity, bias=bias_o[j],
                                         scale=1.0)
                    nc.sync.dma_start(out=outd[128 * j:128 * j + 128, c0:c0 + w],
                                      in_=ysb[0:128, 0:w])

            av_chunk(0, AV_A)
            out_proj(0, 0, 512)
            av_chunk(1, AV_B)
            out_proj(1, 512, 512)
            out_proj(2, 1024, 1)
        if LV < 5:
            for j in range(3):
                nc.sync.dma_start(out=outd[128 * j:128 * j + 128, 0:L],
                                  in_=xT[j][:, FP:FP + L])

    nc.compile()
    return nc


_CACHE = {}


def get_nc():
    if "nc" not in _CACHE:
        nc = bacc.Bacc("TRN2", target_bir_lowering=False, debug=False)
        _CACHE["nc"] = build_program(nc)
    return _CACHE["nc"]


def kernel(x, Wq, bq, Wk, bk, Wv, bv, Wo, bo, _trace=False):
    from concourse.bass_utils import run_bass_kernel_spmd

    x = np.asarray(x)
    in_maps = [
        host_inputs(x[b], Wq, bq, Wk, bk, Wv, bv, Wo, bo) for b in range(B)
    ]
    nc = get_nc()
    res = run_bass_kernel_spmd(nc, in_maps, core_ids=list(range(8)), trace=_trace)
    out = np.stack(
        [np.asarray(res.results[b]["outT"], dtype=np.float32).T for b in range(B)],
        axis=0,
    )
    if _trace:
        return out, res
    return out


# revision 32
# speedup vs baseline: 1.4322x; 1.1330x over previous
"""MultiHeadLocalAttention Trainium2 kernel (v6).

Sharding: data-parallel over batch B=8 across the 8 NeuronCores (one batch
element per core).

v6 design (vs v5): everything is organized around KEY-ALIGNED 128-key tiles
so each banded score is ONE wide matmul, and the AV matmul produces the
attention output TRANSPOSED (O^T, head-dim on partitions) so no PE
transposes are needed and the output projection emits y^T directly
(host un-transposes).

  - scores: per (key tile t, head h): S^T[k,q] = K_t^T Q_span, one matmul
    [64 contraction, 128 keys out, 160 q cols].  8 tiles x 6 heads = 48.
    All Q/K tiles are per-head [64, f] at partition base 0 — mixing 64-row
    weight loads at PE row 0 and row 64 back-to-back crashes TRN2.
  - exp on scalar engine (psum->SBUF), band mask via one tensor_mul with a
    replicated [128, 960] band mask.
  - AV: per (head, 512-q-chunk): oT[128, q] = sum_t Vpp2_t,h^T am_t in
    PSUM.  Vpp2[t] is [128 keys, 768]: per head 64 V columns + 64 constant
    1.0 columns, so oT rows 0:64 = O^T raw and rows 64:128 = the softmax
    denominator REPLICATED 64x.  The cls-key rank-1 term exp(s_cls) x
    [v_cls|ones] runs FIRST with start=True so no psum pre-zeroing is
    needed.
  - normalize: one vector reciprocal [64,512] (psum rows 64:128 -> SBUF)
    + one tensor_mul psum x rrect -> OT[e, q] (bf16, SBUF).
  - out proj: y^T[eo, q] = Wo^T O^T, bias added in the psum drain
    (per-partition bias).  DMA out as [E, L]; host transposes.
  - cls query (q=0) follows v5's scheme (full softmax over all keys).
  - PE p-state: a run of dummy matmuls at t=0 ramps the PE clock while
    the input DMAs stream in.
"""

import os
import sys

sys.path.insert(0, "/opt/trn_rl_repo")

import numpy as np
from contextlib import ExitStack

import concourse.bass as bass
import concourse.tile as tile
from concourse import bacc, mybir

H, D = 6, 64
WIN, PAD = 33, 16
B, L, E = 8, 1025, 384
NT = 8            # 128-key tiles covering keys 1..1024
NV = 8            # V token tiles (tokens 1..1024; pad tiles are all-zero)
FP = 16           # f = abs + FP in xT/QT/KT
KTW = FP + L + 16         # 1057
XTW = FP + L + 128        # 1169
SPAN = 160                # q-span per key tile
V2W = 6 * 128             # 768: per head 64 V cols + 64 ones cols
AMW = 6 * SPAN            # 960
F32 = mybir.dt.float32
BF = mybir.dt.bfloat16
AF = mybir.ActivationFunctionType
ALU = mybir.AluOpType

N_DUMMY = 12      # PE-ramp matmuls while input DMAs run
_ORD = ["1", "21", "22", "23", "24", "3"]
LV = _ORD.index(os.environ.get("KPHASE", "3"))

# AV accumulation slices: (tile, am c0, am c1, psum col start)
AV_A = [(0, 16, 160, 0), (1, 0, 160, 112), (2, 0, 160, 240),
        (3, 0, 144, 368), (4, 0, 16, 496)]
AV_B = [(3, 144, 160, 0), (4, 16, 160, 0), (5, 0, 160, 112),
        (6, 0, 160, 240), (7, 0, 144, 368)]


def host_inputs(x_b, Wq, bq, Wk, bk, Wv, bv, Wo, bo):
    """Per-core input dict (numpy). x_b is this core's [L, E] slice."""
    import ml_dtypes
    bf = ml_dtypes.bfloat16
    scale = 1.0 / np.sqrt(np.float32(D))
    wq = np.asarray(Wq, np.float32) * scale
    bq_s = np.asarray(bq, np.float32) * scale
    bo_eff = (
        np.asarray(bv, np.float32) @ np.asarray(Wo, np.float32)
        + np.asarray(bo, np.float32)
    )

    # smalls [128, 15] f32: col h = bq head h (rows 0:64), col 6+h = bk
    # head h (rows 0:64), col 12+j = bo_eff tile j (128 rows)
    smalls = np.zeros((128, 15), np.float32)
    for h in range(6):
        smalls[0:64, h] = bq_s[64 * h:64 * h + 64]
        smalls[0:64, 6 + h] = np.asarray(bk, np.float32)[64 * h:64 * h + 64]
    for j in range(3):
        smalls[:, 12 + j] = bo_eff[128 * j:128 * j + 128]

    wv = np.asarray(Wv, np.float32)

    # band mask [128, 160]: valid iff 0 <= c - k <= 32
    k = np.arange(128)[:, None]
    c = np.arange(SPAN)[None, :]
    bandm = ((c - k >= 0) & (c - k <= 32)).astype(np.float32)

    ident = np.eye(128, dtype=np.float32)

    # x transposed + f-padded on host: [E, XTW], f = abs_token + FP
    xt = np.zeros((E, XTW), np.float32)
    xt[:, FP:FP + L] = np.asarray(x_b, np.float32).T

    return {
        "xt": np.ascontiguousarray(xt, dtype=bf),
        "wq": np.ascontiguousarray(wq, dtype=bf),
        "wk": np.ascontiguousarray(Wk, dtype=bf),
        "wv": np.ascontiguousarray(wv, dtype=bf),
        "wo": np.ascontiguousarray(Wo, dtype=bf),
        "smalls": np.ascontiguousarray(smalls),
        "ident": np.ascontiguousarray(ident, dtype=bf),
        "bandm": np.ascontiguousarray(bandm, dtype=bf),
    }


def build_program(nc):
    xd = nc.dram_tensor("xt", [E, XTW], BF, kind="ExternalInput").ap()
    wqd = nc.dram_tensor("wq", [E, E], BF, kind="ExternalInput").ap()
    wkd = nc.dram_tensor("wk", [E, E], BF, kind="ExternalInput").ap()
    wvd = nc.dram_tensor("wv", [E, E], BF, kind="ExternalInput").ap()
    wod = nc.dram_tensor("wo", [E, E], BF, kind="ExternalInput").ap()
    smd = nc.dram_tensor("smalls", [128, 15], F32, kind="ExternalInput").ap()
    idd = nc.dram_tensor("ident", [128, 128], BF, kind="ExternalInput").ap()
    bmd = nc.dram_tensor("bandm", [128, SPAN], BF, kind="ExternalInput").ap()
    outd = nc.dram_tensor("outT", [E, L], BF, kind="ExternalOutput").ap()

    with tile.TileContext(nc) as tc, ExitStack() as ctx:
        singles = ctx.enter_context(tc.tile_pool(name="singles", bufs=1))
        aepool = ctx.enter_context(tc.tile_pool(name="aepool", bufs=3))
        rpool = ctx.enter_context(tc.tile_pool(name="rpool", bufs=2))
        ypool = ctx.enter_context(tc.tile_pool(name="ypool", bufs=3))

        # ---- persistent SBUF tensors ----
        xT = [singles.tile([128, XTW], BF, tag=f"xT{j}", name=f"xT{j}")
              for j in range(3)]
        QT = [singles.tile([64, KTW], BF, tag=f"QT{h}", name=f"QT{h}")
              for h in range(6)]
        KT = [singles.tile([64, KTW], BF, tag=f"KT{h}", name=f"KT{h}")
              for h in range(6)]
        Vpp = [singles.tile([128, V2W], BF, tag=f"Vpp{t}", name=f"Vpp{t}")
               for t in range(NV)]
        OT = [singles.tile([128, L], BF, tag=f"OT{j}", name=f"OT{j}")
              for j in range(3)]
        am = [singles.tile([128, AMW], BF, tag=f"am{t}", name=f"am{t}")
              for t in range(NT)]
        mask6 = singles.tile([128, AMW], BF, tag="mask6", name="mask6")
        ecls_a = singles.tile([128, 1024], BF, tag="ecls_a", name="ecls_a")
        ecls_b = singles.tile([64, 1024], BF, tag="ecls_b", name="ecls_b")
        vcls6 = singles.tile([128, 256], BF, tag="vcls6", name="vcls6")
        vclsp = singles.tile([1, E], BF, tag="vclsp", name="vclsp")
        vclsp2 = singles.tile([1, V2W], BF, tag="vclsp2", name="vclsp2")
        cls_ab = singles.tile([128, L], BF, tag="cls_ab", name="cls_ab")
        cls_ab2 = singles.tile([64, L], BF, tag="cls_ab2", name="cls_ab2")
        acls = singles.tile([6, 1184], BF, tag="acls", name="acls")
        aclsT = singles.tile([128, 6 * NV], BF, tag="aclsT", name="aclsT")
        acls0 = singles.tile([1, 6], BF, tag="acls0", name="acls0")
        ocls_a = singles.tile([6, 512], BF, tag="ocls_a", name="ocls_a")
        ocls_b = singles.tile([6, 256], BF, tag="ocls_b", name="ocls_b")
        dw = singles.tile([128, 512], BF, tag="dw", name="dw")
        smalls_sb = singles.tile([128, 15], F32, tag="smalls", name="smalls_sb")
        ident_sb = singles.tile([128, 128], BF, tag="ident", name="ident_sb")
        bandm_sb = singles.tile([128, SPAN], BF, tag="bandm", name="bandm_sb")

        # ---- input DMAs across 3 queues; critical tiles first ----
        wsb = {}
        for nm, dr in (("wq", wqd), ("wk", wkd), ("wv", wvd), ("wo", wod)):
            wsb[nm] = [singles.tile([128, E], BF, tag=f"{nm}{ki}",
                                    name=f"{nm}{ki}") for ki in range(3)]
        for ki in range(3):
            nc.sync.dma_start(out=wsb["wq"][ki][:], in_=wqd[ki * 128:ki * 128 + 128, :])
        XH = 592
        for j in range(3):
            nc.sync.dma_start(out=xT[j][:, 0:XH], in_=xd[j * 128:(j + 1) * 128, 0:XH])
        for j in range(3):
            nc.sync.dma_start(out=xT[j][:, XH:XTW], in_=xd[j * 128:(j + 1) * 128, XH:XTW])
        for ki in range(3):
            nc.scalar.dma_start(out=wsb["wk"][ki][:], in_=wkd[ki * 128:ki * 128 + 128, :])
        for ki in range(3):
            nc.scalar.dma_start(out=wsb["wv"][ki][:], in_=wvd[ki * 128:ki * 128 + 128, :])
        for ki in range(3):
            nc.gpsimd.dma_start(out=wsb["wo"][ki][:], in_=wod[ki * 128:ki * 128 + 128, :])
        nc.gpsimd.dma_start(out=smalls_sb[:], in_=smd[:])
        nc.gpsimd.dma_start(out=ident_sb[:], in_=idd[:])
        nc.gpsimd.dma_start(out=bandm_sb[:], in_=bmd[:])

        # ---- immediate on-chip init (no DMA deps) ----
        nc.vector.memset(dw[:], 0.0)
        for h in range(6):
            nc.vector.memset(QT[h][:, 0:FP], 0.0)
            nc.vector.memset(QT[h][:, FP + L:KTW], 0.0)
        nc.vector.memset(acls[:, 0:16], 0.0)
        nc.vector.memset(acls[:, 1040:1184], 0.0)
        # ones columns of the V tiles and of vcls6 are constant 1.0
        for t in range(NV):
            nc.gpsimd.memset(Vpp[t][:], 1.0)
        nc.gpsimd.memset(vcls6[:], 1.0)
        nc.gpsimd.memset(vclsp2[:], 1.0)
        # mask6 = band mask replicated per head (after bandm arrives)
        for h in range(6):
            nc.gpsimd.tensor_copy(mask6[:, SPAN * h:SPAN * h + SPAN], bandm_sb[:])

        bias_q = [smalls_sb[0:64, h:h + 1] for h in range(6)]
        bias_k = [smalls_sb[0:64, 6 + h:7 + h] for h in range(6)]
        bias_o = [smalls_sb[:, 12 + j:13 + j] for j in range(3)]

        # === single PSUM pool: big [128,1024] x3 (6 banks) + aux x2 = 8 ===
        ps = ctx.enter_context(tc.tile_pool(name="ps", bufs=1, space="PSUM"))

        def big_psum():
            return ps.tile([128, 1024], F32, tag="big", name="big", bufs=3)

        def aux_psum(p=128, dt=F32):
            return ps.tile([p, 512], dt, tag="aux", name="aux", bufs=2)

        # =========== phase 1: projections (Q/K/V interleaved) ===========
        if True:
            def v_tile(t):
                pv = aux_psum()
                for ki in range(3):
                    nc.tensor.matmul(
                        pv[:, 0:E],
                        lhsT=xT[ki][:, FP + 1 + 128 * t: FP + 129 + 128 * t],
                        rhs=wsb["wv"][ki][:],
                        start=(ki == 0), stop=(ki == 2),
                    )
                nc.vector.tensor_copy(
                    Vpp[t][:].rearrange("p (h c) -> p h c", h=6)[:, :, 64:128],
                    pv[:, 0:E].rearrange("p (h c) -> p h c", h=6),
                )

            first = True
            for j in range(3):
                for nm, dest, bias in (("wq", QT, bias_q), ("wk", KT, bias_k)):
                    pp = big_psum()
                    if first:
                        # PE p-state ramp: harmless dummy matmuls into the
                        # first psum tile while input DMAs stream in.
                        for _ in range(N_DUMMY):
                            nc.tensor.matmul(pp[0:128, 0:512],
                                             lhsT=dw[:, 0:128], rhs=dw[:],
                                             start=True, stop=True)
                        first = False
                    for c0 in (0, 512):
                        for ki in range(3):
                            nc.tensor.matmul(
                                pp[0:128, c0:c0 + 512],
                                lhsT=wsb[nm][ki][:, 128 * j:128 * j + 128],
                                rhs=xT[ki][:, FP + c0: FP + c0 + 512],
                                start=(ki == 0), stop=(ki == 2),
                            )
                    pt_t = aux_psum()
                    for ki in range(3):
                        nc.tensor.matmul(
                            pt_t[0:128, 0:1],
                            lhsT=wsb[nm][ki][:, 128 * j:128 * j + 128],
                            rhs=xT[ki][:, FP + 1024: FP + 1025],
                            start=(ki == 0), stop=(ki == 2),
                        )
                    for par in range(2):
                        hh = 2 * j + par
                        if par == 0:
                            nc.scalar.activation(
                                out=dest[hh][:, FP:FP + 1024],
                                in_=pp[0:64, 0:1024],
                                func=AF.Identity, bias=bias[hh], scale=1.0,
                            )
                            nc.scalar.activation(
                                out=dest[hh][:, FP + 1024:FP + 1025],
                                in_=pt_t[0:64, 0:1],
                                func=AF.Identity, bias=bias[hh], scale=1.0,
                            )
                        else:
                            nc.vector.tensor_scalar_add(
                                dest[hh][:, FP:FP + 1024],
                                pp[64:128, 0:1024], bias[hh],
                            )
                            nc.vector.tensor_scalar_add(
                                dest[hh][:, FP + 1024:FP + 1025],
                                pt_t[64:128, 0:1], bias[hh],
                            )
                # V tiles interleaved behind each (Q,K) pair
                for t in range(3 * j, min(3 * j + 3, NV)):
                    v_tile(t)

            # vcls = V row of token 0
            pvc = aux_psum()
            for ki in range(3):
                nc.tensor.matmul(
                    pvc[0:1, 0:E], lhsT=xT[ki][:, FP:FP + 1],
                    rhs=wsb["wv"][ki][:], start=(ki == 0), stop=(ki == 2),
                )
            nc.vector.tensor_copy(vclsp[:], pvc[0:1, 0:E])
            # scatter vcls to partitions 32*(h%4), cols 128*(h//4)+64[:128]
            for h in range(6):
                nc.sync.dma_start(
                    out=vcls6[32 * (h % 4):32 * (h % 4) + 1,
                              128 * (h // 4) + 64:128 * (h // 4) + 128],
                    in_=vclsp[0:1, 64 * h:64 * h + 64],
                )
            # vclsp2: token-0 V row in Vpp2 layout (V blocks strided)
            nc.sync.dma_start(
                out=vclsp2[0:1, :].rearrange("p (h c) -> p h c", h=6)[:, :, 64:128],
                in_=vclsp[0:1, :].rearrange("p (h c) -> p h c", h=6),
            )

        # =========== phase 2+3 ===========
        # order: ecls -> banded scores -> cls-query -> AV-A -> cls tail
        #        -> AV-B -> out proj
        if LV >= 2:
            # --- cls-KEY scores: s_cls[h, q] for q=1..1024 ---
            for ci in range(2):
                pe_a = aux_psum()
                pe_b = aux_psum(p=64)
                qs0 = FP + 1 + 512 * ci
                nc.tensor.matmul(pe_a[0:128, 0:512], lhsT=dw[0:1, 0:128],
                                 rhs=dw[0:1, 0:512], start=True, stop=True)
                nc.tensor.matmul(pe_b[0:64, 0:512], lhsT=dw[0:1, 0:64],
                                 rhs=dw[0:1, 0:512], start=True, stop=True)
                for h in range(6):
                    dstp = pe_a if h < 4 else pe_b
                    base = 32 * (h % 4)
                    nc.tensor.matmul(
                        dstp[base:base + 1, 0:512],
                        lhsT=KT[h][0:64, FP:FP + 1],
                        rhs=QT[h][0:64, qs0:qs0 + 512],
                        start=True, stop=True, tile_position=(0, base),
                    )
                nc.scalar.activation(
                    out=ecls_a[:, 512 * ci:512 * ci + 512],
                    in_=pe_a[:], func=AF.Exp)
                nc.scalar.activation(
                    out=ecls_b[:, 512 * ci:512 * ci + 512],
                    in_=pe_b[:], func=AF.Exp)

        if LV >= 3:
            # --- banded scores per key tile ---
            for t in range(NT):
                scp = big_psum()
                k0 = FP + 1 + 128 * t
                q0 = FP - 15 + 128 * t
                for h in range(6):
                    off = 512 * (h // 3) + SPAN * (h % 3)
                    nc.tensor.matmul(
                        scp[:, off:off + SPAN],
                        lhsT=KT[h][0:64, k0:k0 + 128],
                        rhs=QT[h][0:64, q0:q0 + SPAN],
                        start=True, stop=True,
                    )
                a_e = aepool.tile([128, AMW], BF, tag="a_e", name="a_e")
                nc.scalar.activation(out=a_e[:, 0:480], in_=scp[:, 0:480],
                                     func=AF.Exp)
                nc.scalar.activation(out=a_e[:, 480:960], in_=scp[:, 512:992],
                                     func=AF.Exp)
                nc.gpsimd.tensor_mul(am[t][:], a_e[:], mask6[:])

        if LV >= 4:
            # --- cls-query scores: s[h, k] for q=0 over all keys ---
            CCH = [(0, 512), (512, 512), (1024, 1)]
            for c0, w in CCH:
                pa = aux_psum()
                pb = aux_psum()
                # init full tiles via zero outer-product (dw is zeros)
                nc.tensor.matmul(pa[0:128, 0:w], lhsT=dw[0:1, 0:128],
                                 rhs=dw[0:1, 0:w], start=True, stop=True)
                nc.tensor.matmul(pb[0:128, 0:w], lhsT=dw[0:1, 0:128],
                                 rhs=dw[0:1, 0:w], start=True, stop=True)
                for h in range(6):
                    dstp = pa if h < 4 else pb
                    base = 32 * (h % 4)
                    nc.tensor.matmul(
                        dstp[base:base + 1, 0:w],
                        lhsT=QT[h][0:64, FP:FP + 1],
                        rhs=KT[h][0:64, FP + c0:FP + c0 + w],
                        start=True, stop=True, tile_position=(0, base),
                    )
                nc.scalar.activation(out=cls_ab[:, c0:c0 + w],
                                     in_=pa[:, 0:w], func=AF.Exp)
                nc.scalar.activation(out=cls_ab2[:, c0:c0 + w],
                                     in_=pb[0:64, 0:w], func=AF.Exp)
            for h in range(6):
                srct = cls_ab if h < 4 else cls_ab2
                nc.sync.dma_start(
                    out=acls[h:h + 1, 15:15 + L],
                    in_=srct[32 * (h % 4):32 * (h % 4) + 1, 0:L],
                )

        def cls_tail():
            for t in range(NV):
                pt = aux_psum(dt=BF)
                nc.tensor.transpose(
                    pt[0:128, 0:6], acls[0:6, 128 * t + 16:128 * t + 144],
                    ident_sb[0:6, 0:6],
                )
                nc.vector.tensor_copy(aclsT[:, 6 * t:6 * t + 6], pt[0:128, 0:6])
            pt0 = aux_psum(dt=BF)
            nc.tensor.transpose(pt0[0:1, 0:6], acls[0:6, 15:16],
                                ident_sb[0:6, 0:6])
            nc.vector.tensor_copy(acls0[:], pt0[0:1, 0:6])

            # cls AV over Vpp2 (two col halves); sums ride any ones col
            poc_a = aux_psum()
            poc_b = aux_psum(p=64)
            nc.tensor.matmul(poc_a[0:6, 0:512], lhsT=acls0[0:1, :],
                             rhs=vclsp2[0:1, 0:512], start=True, stop=False)
            nc.tensor.matmul(poc_b[0:6, 0:256], lhsT=acls0[0:1, :],
                             rhs=vclsp2[0:1, 512:768], start=True, stop=False)
            for t in range(NV):
                nc.tensor.matmul(
                    poc_a[0:6, 0:512], lhsT=aclsT[:, 6 * t:6 * t + 6],
                    rhs=Vpp[t][:, 0:512], start=False, stop=(t == NV - 1),
                )
                nc.tensor.matmul(
                    poc_b[0:6, 0:256], lhsT=aclsT[:, 6 * t:6 * t + 6],
                    rhs=Vpp[t][:, 512:768], start=False, stop=(t == NV - 1),
                )
            rc = singles.tile([6, 1], F32, tag="rc", name="rc")
            nc.vector.reciprocal(rc[:], poc_a[0:6, 0:1])
            nc.vector.tensor_scalar_mul(ocls_a[:], poc_a[0:6, 0:512], rc[:])
            nc.vector.tensor_scalar_mul(ocls_b[:], poc_b[0:6, 0:256], rc[:])
            for h in range(6):
                srco = ocls_a if h < 4 else ocls_b
                c0 = (128 * h if h < 4 else 128 * (h - 4)) + 64
                nc.sync.dma_start(
                    out=OT[h // 2][64 * (h % 2):64 * (h % 2) + 64, 0:1],
                    in_=srco[h:h + 1, c0:c0 + 64],
                )

        # =========== phase 3: AV + normalize + out proj ===========
        if LV >= 5:
            def av_chunk(ci, slices):
                for hp in range(3):
                    pair = big_psum()
                    for par in range(2):
                        h = 2 * hp + par
                        po = pair[:, 512 * par:512 * par + 512]
                        esrc = ecls_a if h < 4 else ecls_b
                        erow = 32 * (h % 4)
                        # cls-key rank-1 first: start=True zeroes the half
                        nc.tensor.matmul(
                            po[0:128, 0:512],
                            lhsT=vcls6[erow:erow + 1,
                                       128 * (h // 4):128 * (h // 4) + 128],
                            rhs=esrc[erow:erow + 1, 512 * ci:512 * ci + 512],
                            start=True, stop=False, tile_position=(erow, 0),
                        )
                        for si, (t, c0, c1, p0) in enumerate(slices):
                            nc.tensor.matmul(
                                po[0:128, p0:p0 + (c1 - c0)],
                                lhsT=Vpp[t][:, 128 * h:128 * h + 128],
                                rhs=am[t][:, SPAN * h + c0:SPAN * h + c1],
                                start=False, stop=(si == len(slices) - 1),
                            )
                        # normalize: r = 1/sums (rows 0:64 = ones block)
                        rrect = rpool.tile([64, 512], F32, tag="rrect",
                                           name="rrect")
                        nc.vector.reciprocal_approx_fast(
                            out=rrect[:], in_=po[0:64, 0:512])
                        nc.vector.tensor_mul(
                            OT[h // 2][64 * (h % 2):64 * (h % 2) + 64,
                                       1 + 512 * ci:513 + 512 * ci],
                            po[64:128, 0:512], rrect[:],
                        )

            def out_proj(ci, c0, w):
                for j in range(3):
                    py = aux_psum()
                    for ki in range(3):
                        nc.tensor.matmul(
                            py[0:128, 0:w],
                            lhsT=wsb["wo"][ki][:, 128 * j:128 * j + 128],
                            rhs=OT[ki][:, c0:c0 + w],
                            start=(ki == 0), stop=(ki == 2),
                        )
                    ysb = ypool.tile([128, 512], BF, tag="ysb", name="ysb")
                    nc.scalar.activation(out=ysb[0:128, 0:w], in_=py[0:128, 0:w],
                                         func=AF.Identity, bias=bias_o[j],
                                         scale=1.0)
                    nc.sync.dma_start(out=outd[128 * j:128 * j + 128, c0:c0 + w],
                                      in_=ysb[0:128, 0:w])

            av_chunk(0, AV_A)
            cls_tail()
            av_chunk(1, AV_B)
            out_proj(0, 0, 512)
            out_proj(1, 512, 512)
            out_proj(2, 1024, 1)
        if LV < 5:
            for j in range(3):
                nc.sync.dma_start(out=outd[128 * j:128 * j + 128, 0:L],
                                  in_=xT[j][:, FP:FP + L])

    nc.compile()
    return nc


_CACHE = {}


def get_nc():
    if "nc" not in _CACHE:
        nc = bacc.Bacc("TRN2", target_bir_lowering=False, debug=False)
        _CACHE["nc"] = build_program(nc)
    return _CACHE["nc"]


def kernel(x, Wq, bq, Wk, bk, Wv, bv, Wo, bo, _trace=False):
    from concourse.bass_utils import run_bass_kernel_spmd

    x = np.asarray(x)
    in_maps = [
        host_inputs(x[b], Wq, bq, Wk, bk, Wv, bv, Wo, bo) for b in range(B)
    ]
    nc = get_nc()
    res = run_bass_kernel_spmd(nc, in_maps, core_ids=list(range(8)), trace=_trace)
    out = np.stack(
        [np.asarray(res.results[b]["outT"], dtype=np.float32).T for b in range(B)],
        axis=0,
    )
    if _trace:
        return out, res
    return out
